# revision 2
# baseline (speedup 1.0000x reference)
"""Multi-head attention kernel for 8 TRN2 NeuronCores.

Problem: b=2, n=2048, d=1024, heads=16, hd=64.
  q/k/v = x @ W{q,k,v}.T (+ zero bias)
  per head: softmax(q k^T / sqrt(d)) @ v
  out = concat @ Wo.T (+ zero bias)

Sharding (8 cores): data-parallel over batch (2) x tensor-parallel over
heads (16 heads -> 4 groups of 4). Core c handles batch c//4, heads
4*(c%4) .. 4*(c%4)+3 (feature slice of 256 columns). Wo is applied
row-parallel: each core emits a partial output; the host sums the 4
partials per batch (and untransposes). No collectives needed.

All matmuls run in float32r (TF32-like). Raw fp32 bits are DMA'd
directly into f32r tiles; on-chip f32r operands are written by rounding
copy/activation producers as the walrus verifier requires.

Key structure decisions:
 - everything is pre-transposed on the host so the kernel needs zero
   on-device transposes: xT (d,n), wqT/wkT/wvT (d,256), woT (256,d).
 - Q^T/K^T [feat, n] via dc-outer accumulation streaming behind the
   xT DMA; V in natural [n, feat] layout with a ones column appended
   (the ones column accumulates the softmax denominators during AV).
 - scores are computed for a HEAD PAIR concurrently via PE row tiling:
   head 2fc lives on partitions 0:64, head 2fc+1 on 64:128 (both in
   QT/KT layout), so the two K=64 matmuls auto-derive tile_position
   (0,0) and (64,0) and run in different row groups of the array at
   the same time -- 2x over the old zero-padded K=128 stationary.
 - passes are (head-pair, q-quarter of 512): per k-chunk, the pair's
   scores land in one 2-bank PSUM tile, ONE exp activation covers both
   heads [128, 2, 512], and two AV matmuls accumulate [65, 512] per
   head (1 PSUM bank each).
 - normalize: copy avo out of PSUM fast (frees the accumulator), then
   reciprocal in a partition-scattered layout, partition_broadcast on
   GpSimd, multiply on DVE.
 - output projection runs per q-quarter as soon as the quarter's last
   heads are normalized (woT stationary, emits partial^T [d, n]).

Biases are structurally zero in this problem spec and are skipped.
"""

import numpy as np

HEADS = 16
D = 1024
N = 2048
B = 2
N_CORES = 8
HPC = HEADS // (N_CORES // B)  # heads per core = 4
HD = D // HEADS                # 64
F = HPC * HD                   # 256 features per core
P = 128


def build_nc(n=N, d=D, hpc=HPC, hd=HD):
    """Build the per-core Bass program (SPMD: same program on all 8 cores)."""
    import concourse.bass as bass
    import concourse.tile as tile
    from concourse import bacc, mybir

    f32 = mybir.dt.float32
    f32r = mybir.dt.float32r
    f = hpc * hd            # per-core feature count (256)
    FC = f // P             # feature chunks (2)
    DC = d // P             # contraction chunks over d (8)
    NT = n // P             # n tiles / k chunks (16)
    QB = 512                # matmul moving block
    SCW = 512               # per-pass q width (quarter)
    NQQ = n // SCW          # q-quarters (4)
    scale = 1.0 / float(np.sqrt(np.float32(d)))

    nc = bacc.Bacc("TRN2")

    xT = nc.declare_dram_parameter("xT", [d, n], f32r, isOutput=False)
    wqT = nc.declare_dram_parameter("wqT", [d, f], f32r, isOutput=False)
    wkT = nc.declare_dram_parameter("wkT", [d, f], f32r, isOutput=False)
    wvT = nc.declare_dram_parameter("wvT", [d, f], f32r, isOutput=False)
    woT = nc.declare_dram_parameter("woT", [f, d], f32r, isOutput=False)
    out = nc.declare_dram_parameter("out", [d, n], f32, isOutput=True)

    xT_c = xT.rearrange("(c p) n -> c p n", p=P)
    wqT_c = wqT.rearrange("(c p) f -> c p f", p=P)
    wkT_c = wkT.rearrange("(c p) f -> c p f", p=P)
    wvT_c = wvT.rearrange("(c p) f -> c p f", p=P)
    woT_c = woT.rearrange("(c p) n -> c p n", p=P)

    with tile.TileContext(nc) as tc:
        with (
            tc.tile_pool(name="qkv", bufs=1) as qkv,
            tc.tile_pool(name="outT", bufs=1) as outp,
            # phase-2 pools created before the phase-1 pools so their
            # SBUF/PSUM ranges are disjoint: early heads' attention overlaps
            # the fc=1 projections with no pool-reuse serialization
            tc.tile_pool(name="pt", bufs=2) as ptp,
            tc.tile_pool(name="norm", bufs=2) as normp,
            tc.tile_pool(name="scps", bufs=2, space="PSUM") as scps,
            tc.tile_pool(name="avps", bufs=1, space="PSUM") as avps,
        ):
            QT_sb = qkv.tile([P, FC, n], f32r)
            KT_sb = qkv.tile([P, FC, n], f32r)
            V_sb = qkv.tile([P, NT, hpc, hd + 1], f32r)
            outT_sb = outp.tile([P, FC, n], f32r)
            # ones column of V_aug: memset f32 const, then write via a
            # rounding DVE copy (direct memset on f32r fails walrus codegen,
            # and f32r matmul operands need rounding writers)
            ones_c = outp.tile([P, 1], f32)
            nc.vector.memset(ones_c[:], 1.0)
            nc.vector.tensor_copy(
                V_sb[:, :, :, hd : hd + 1],
                ones_c.to_broadcast([P, NT, hpc, 1]),
            )

            def pass_begin():
                avoA = avps.tile([hd + 1, SCW], f32, tag="avoA", name="avoA")
                avoB = avps.tile([hd + 1, SCW], f32, tag="avoB", name="avoB")
                return avoA, avoB

            def pass_blocks(avos, hp, qq, kcs, pre_kc=None):
                """head-pair scores^T (row-tiled) -> exp -> 2x AV accumulate
                for k-chunks `kcs`."""
                avoA, avoB = avos
                q0 = qq * SCW
                for kc in kcs:
                    if pre_kc is not None:
                        pre_kc(kc)
                    sc = scps.tile([P, 2, SCW], f32, tag="sc")
                    ks = slice(kc * P, (kc + 1) * P)
                    qs = slice(q0, q0 + SCW)
                    # two K=64 matmuls in different PE row groups run
                    # concurrently (tile_position (0,0) / (64,0) auto-derived
                    # from the operand base partitions)
                    nc.tensor.matmul(
                        sc[:, 0, :],
                        KT_sb[0:hd, hp, ks],
                        QT_sb[0:hd, hp, qs],
                        start=True,
                        stop=True,
                    )
                    nc.tensor.matmul(
                        sc[:, 1, :],
                        KT_sb[hd : 2 * hd, hp, ks],
                        QT_sb[hd : 2 * hd, hp, qs],
                        start=True,
                        stop=True,
                    )
                    pt = ptp.tile([P, 2, SCW], f32r, tag="pt")
                    nc.scalar.activation(
                        pt[:], sc[:], mybir.ActivationFunctionType.Exp,
                        scale=scale,
                    )
                    nc.tensor.matmul(
                        avoA[:],
                        V_sb[:, kc, 2 * hp, :],
                        pt[:, 0, :],
                        start=(kc == 0),
                        stop=(kc == NT - 1),
                    )
                    nc.tensor.matmul(
                        avoB[:],
                        V_sb[:, kc, 2 * hp + 1, :],
                        pt[:, 1, :],
                        start=(kc == 0),
                        stop=(kc == NT - 1),
                    )

            def pass_end(avo, h, qq):
                """Free avo fast, then normalize rows 0..hd-1 by row hd (the
                softmax sums). reciprocal is single-lane-slow on a [1, SCW]
                row, so scatter the sums across partitions via a small SBUF
                DMA round-trip first."""
                fc = (h * hd) // P
                po = (h * hd) % P
                q0 = qq * SCW
                av_sb = normp.tile([hd + 1, SCW], f32, tag="av_sb")
                nc.vector.tensor_copy(av_sb[:], avo[:])
                rsh = normp.tile([P, SCW // P], f32, tag="rsh")
                nc.sync.dma_start(out=rsh[:], in_=av_sb[hd : hd + 1, :])
                rsh2 = normp.tile([P, SCW // P], f32, tag="rsh2")
                nc.vector.reciprocal(rsh2[:], rsh[:])
                recip = normp.tile([1, SCW], f32, tag="recip")
                nc.sync.dma_start(out=recip[:], in_=rsh2[:])
                bc = normp.tile([hd, SCW], f32, tag="bc")
                nc.gpsimd.partition_broadcast(bc[:], recip[:])
                nc.vector.tensor_mul(
                    outT_sb[po : po + hd, fc, q0 : q0 + SCW],
                    av_sb[0:hd, :],
                    bc[:],
                )

            def do_pass(hp, qq, pre_kc=None):
                avos = pass_begin()
                pass_blocks(avos, hp, qq, range(NT), pre_kc=pre_kc)
                pass_end(avos[0], 2 * hp, qq)
                pass_end(avos[1], 2 * hp + 1, qq)

            # ---- Phase 1 + head-pair 0 passes, emission-interleaved ----
            with (
                tc.tile_pool(name="xw", bufs=1) as xw,
                tc.tile_pool(name="p1ps", bufs=2, space="PSUM") as p1ps,
            ):
                xT_r = xw.tile([P, DC, n], f32r)
                wqT_r = xw.tile([P, DC, f], f32r)
                wkT_r = xw.tile([P, DC, f], f32r)
                wvT_r = xw.tile([P, DC, f], f32r)

                # wq + xT interleaved per chunk: QT matmuls stream right
                # behind them; wk/wv stream during QT/KT compute.
                for dc in range(DC):
                    nc.sync.dma_start(out=wqT_r[:, dc, :], in_=wqT_c[dc])
                    nc.sync.dma_start(out=xT_r[:, dc, :], in_=xT_c[dc])

                def proj_cols(w_sb, dst, fc, qcp):
                    # dc-outer accumulation, one sub-stage of 2 held banks
                    # covering moving columns [qcp*QB, (qcp+2)*QB)
                    pss = [
                        p1ps.tile([P, QB], f32, tag="big", name=f"pj{g}")
                        for g in range(2)
                    ]
                    for dc in range(DC):
                        for j in range(2):
                            qc = qcp + j
                            nc.tensor.matmul(
                                pss[j][:],
                                w_sb[:, dc, fc * P : (fc + 1) * P],
                                xT_r[:, dc, qc * QB : (qc + 1) * QB],
                                start=(dc == 0),
                                stop=(dc == DC - 1),
                            )
                    for j in range(2):
                        qc = qcp + j
                        sl = slice(qc * QB, (qc + 1) * QB)
                        nc.vector.tensor_copy(dst[:, fc, sl], pss[j][:])

                def v_tile(nt):
                    ps = p1ps.tile([P, QB], f32, tag="big", name="vps")
                    for dc in range(DC):
                        nc.tensor.matmul(
                            ps[:, 0:f],
                            xT_r[:, dc, nt * P : (nt + 1) * P],
                            wvT_r[:, dc, :],
                            start=(dc == 0),
                            stop=(dc == DC - 1),
                        )
                    nc.vector.tensor_copy(
                        V_sb[:, nt, :, 0:hd],
                        ps[:, 0:f].rearrange("p (h e) -> p h e", h=hpc),
                    )

                # wk needed right after the first k0 sub-stage; wv by the
                # first v_tile — both AFTER the xT stream in queue order so
                # they don't delay the projection-gating xT chunks
                for dc in range(DC):
                    nc.sync.dma_start(out=wkT_r[:, dc, :], in_=wkT_c[dc])
                for dc in range(DC):
                    nc.sync.dma_start(out=wvT_r[:, dc, :], in_=wvT_c[dc])
                # Emission order = scheduling priority. Minimal chain to the
                # first exp: QT cols 0:1024 of fc0, then KT fc0 in column
                # sub-stages interleaved with the first pass's blocks (V
                # tiles interleaved per k-chunk they feed). Later projections
                # are emitted after the passes they should yield priority
                # to, so they fill the PE's slack.
                proj_cols(wqT_r, QT_sb, 0, 0)  # QT fc0 cols 0:1024 (qq 0,1)
                avos0 = pass_begin()
                proj_cols(wkT_r, KT_sb, 0, 0)  # KT fc0 cols 0:1024 (kc 0..7)
                pass_blocks(avos0, 0, 0, range(0, NT // 2), pre_kc=v_tile)
                proj_cols(wkT_r, KT_sb, 0, 2)  # KT fc0 cols 1024:2048
                pass_blocks(avos0, 0, 0, range(NT // 2, NT), pre_kc=v_tile)
                pass_end(avos0[0], 0, 0)
                pass_end(avos0[1], 1, 0)
                do_pass(0, 1)
                proj_cols(wqT_r, QT_sb, 0, 2)  # QT fc0 cols for qq 2,3
                do_pass(0, 2)
                do_pass(0, 3)
                proj_cols(wqT_r, QT_sb, 1, 0)
                proj_cols(wqT_r, QT_sb, 1, 2)
                proj_cols(wkT_r, KT_sb, 1, 0)
                proj_cols(wkT_r, KT_sb, 1, 2)

            # ---- head-pair 1 passes + per-q-quarter output projection ----
            with (
                tc.tile_pool(name="wo", bufs=1) as wop,
                tc.tile_pool(name="wops", bufs=2, space="PSUM") as wopsp,
                tc.tile_pool(name="wosb", bufs=4) as wosbp,
            ):
                woT_sb = wop.tile([P, FC, d], f32r)
                for fc in range(FC):
                    nc.sync.dma_start(out=woT_sb[:, fc, :], in_=woT_c[fc])

                def wo_quarter(qq):
                    # output projection for q-quarter qq (woT stationary;
                    # emits partial^T [d, n])
                    q0 = qq * SCW
                    for do in range(d // P):
                        ps = wopsp.tile([P, SCW], f32, tag="wops")
                        for fc in range(FC):
                            nc.tensor.matmul(
                                ps[:],
                                woT_sb[:, fc, do * P : (do + 1) * P],
                                outT_sb[:, fc, q0 : q0 + SCW],
                                start=(fc == 0),
                                stop=(fc == FC - 1),
                            )
                        ob = wosbp.tile([P, SCW], f32, tag="ob")
                        nc.vector.tensor_copy(ob[:], ps[:])
                        nc.sync.dma_start(
                            out=out[
                                do * P : (do + 1) * P,
                                q0 : q0 + SCW,
                            ],
                            in_=ob[:],
                        )

                do_pass(1, 0)
                wo_quarter(0)
                do_pass(1, 1)
                wo_quarter(1)
                do_pass(1, 2)
                wo_quarter(2)
                do_pass(1, 3)
                wo_quarter(3)
    nc.finalize()
    return nc


def make_in_maps(x, Wq, Wk, Wv, Wo):
    """Shard full inputs into per-core DRAM parameter maps."""
    x = np.asarray(x, dtype=np.float32)
    Wq = np.asarray(Wq, dtype=np.float32)
    Wk = np.asarray(Wk, dtype=np.float32)
    Wv = np.asarray(Wv, dtype=np.float32)
    Wo = np.asarray(Wo, dtype=np.float32)
    xTs = [np.ascontiguousarray(x[b].T) for b in range(B)]
    WqT, WkT, WvT = Wq.T, Wk.T, Wv.T
    in_maps = []
    for c in range(N_CORES):
        b, g = c // (N_CORES // B), c % (N_CORES // B)
        fs = slice(g * F, (g + 1) * F)
        in_maps.append(
            {
                "xT": xTs[b],
                "wqT": np.ascontiguousarray(WqT[:, fs]),
                "wkT": np.ascontiguousarray(WkT[:, fs]),
                "wvT": np.ascontiguousarray(WvT[:, fs]),
                "woT": np.ascontiguousarray(Wo[:, fs].T),
            }
        )
    return in_maps


_NC_CACHE = {}


def _enable_ldw_opt():
    """Flip walrus --enable-ldw-opt to true: consecutive matmuls sharing a
    stationary operand skip the redundant LDWEIGHTS reload."""
    import concourse.bass_utils as bu

    if getattr(bu, "_ldw_opt_patched", False):
        return
    orig = bu.run_command

    def patched(argv, **kw):
        argv = [
            "--enable-ldw-opt=true" if a == "--enable-ldw-opt=false" else a
            for a in argv
        ]
        return orig(argv, **kw)

    bu.run_command = patched
    bu._ldw_opt_patched = True


def run(x, Wq, Wk, Wv, Wo, trace=False):
    from concourse.bass_utils import run_bass_kernel_spmd

    _enable_ldw_opt()
    if "nc" not in _NC_CACHE:
        _NC_CACHE["nc"] = build_nc()
    nc = _NC_CACHE["nc"]
    in_maps = make_in_maps(x, Wq, Wk, Wv, Wo)
    res = run_bass_kernel_spmd(nc, in_maps, core_ids=list(range(N_CORES)), trace=trace)
    parts = [np.asarray(res.results[i]["out"]) for i in range(N_CORES)]
    gpb = N_CORES // B
    # per-core partials are transposed [d, n]: sum the group, then untranspose
    full = np.stack(
        [
            sum(parts[b * gpb + 1 : (b + 1) * gpb], parts[b * gpb]).T
            for b in range(B)
        ]
    )
    return np.ascontiguousarray(full, dtype=np.float32), res


def kernel(x, Wq, bq, Wk, bk, Wv, bv, Wo, bo):
    full, _ = run(x, Wq, Wk, Wv, Wo)
    return full


# revision 4
# speedup vs baseline: 1.3058x; 1.3058x over previous
"""Multi-head attention kernel for 8 TRN2 NeuronCores.

Problem: b=2, n=2048, d=1024, heads=16, hd=64.
  q/k/v = x @ W{q,k,v}.T (+ zero bias)
  per head: softmax(q k^T / sqrt(d)) @ v
  out = concat @ Wo.T (+ zero bias)

Sharding (8 cores): data-parallel over batch (2) x tensor-parallel over
heads (16 heads -> 4 groups of 4). Core c handles batch c//4, heads
4*(c%4) .. 4*(c%4)+3 (feature slice of 256 columns). Wo is applied
row-parallel: each core emits a partial output; the host sums the 4
partials per batch (and untransposes). No collectives needed.

Attention matmuls run in float32r (TF32-like: full PE rate with moving
free dim >=256). Phase-1 inputs (xT and the q/k/v weights) are bf16:
same PE rate, but half the HBM bytes -- the xT stream is what gates the
kernel start. Raw input bits are DMA'd directly into same-dtype tiles;
on-chip f32r matmul operands are written by rounding copy/activation
producers as the walrus verifier requires.

Key structure decisions (all measured on HW):
 - everything is pre-transposed on the host so the kernel needs zero
   on-device transposes: xT (d,n), wqT/wkT/wvT (d,256), woT (256,d).
 - Q^T/K^T [feat, n] via dc-outer accumulation streaming behind the
   xT DMA; V in natural [n, feat] layout with a ones column appended
   (the ones column accumulates the softmax denominators during AV).
 - K^T is stored zero-padded per head to a full 128-row stationary:
   naked K=64 matmuls (even row-tiled concurrent pairs -- measured)
   make the HAM read low PE activity and clock-gate the array to half
   speed for the whole kernel; zero-padded K=128 runs at 1 cyc/row.
 - passes are (head-pair, q-quarter of 512): per k-chunk both heads'
   scores^T [k, q] land in one 2-bank PSUM tile, ONE exp activation
   covers the pair [128, 2, 512] (ScalarE is the pacing floor of the
   inner loop: n*n*heads/core exps at 1 elem/cycle/lane @1.2GHz), and
   two AV matmuls accumulate [65, q] per head (1 PSUM bank each).
 - pass emission is software-pipelined: the AV matmuls for k-chunk kc
   are emitted after the scores for kc+1, so the PE instruction stream
   has ready work while the exp for kc is still in flight.
 - normalize: copy avo out of PSUM fast (frees the accumulator), then
   reciprocal in a partition-scattered layout (a [1, n] row reciprocal
   is single-lane and 60x slower), partition_broadcast on GpSimd,
   multiply on DVE.
 - output projection runs per q-quarter right after the quarter's last
   heads normalize (woT stationary, emits the partial TRANSPOSED
   [d, n]; the host untransposes).

Biases are structurally zero in this problem spec and are skipped.
"""

import numpy as np

HEADS = 16
D = 1024
N = 2048
B = 2
N_CORES = 8
HPC = HEADS // (N_CORES // B)  # heads per core = 4
HD = D // HEADS                # 64
F = HPC * HD                   # 256 features per core
P = 128


def build_nc(n=N, d=D, hpc=HPC, hd=HD):
    """Build the per-core Bass program (SPMD: same program on all 8 cores)."""
    import concourse.bass as bass
    import concourse.tile as tile
    from concourse import bacc, mybir

    f32 = mybir.dt.float32
    f32r = mybir.dt.float32r
    bf16 = mybir.dt.bfloat16
    f = hpc * hd            # per-core feature count (256)
    FC = f // P             # feature chunks (2)
    DC = d // P             # contraction chunks over d (8)
    NT = n // P             # n tiles / k chunks (16)
    QB = 512                # matmul moving block
    SCW = 512               # per-pass q width (quarter)
    NQQ = n // SCW          # q-quarters (4)
    scale = 1.0 / float(np.sqrt(np.float32(d)))

    nc = bacc.Bacc("TRN2")

    xT = nc.declare_dram_parameter("xT", [d, n], bf16, isOutput=False)
    wqT = nc.declare_dram_parameter("wqT", [d, f], bf16, isOutput=False)
    wkT = nc.declare_dram_parameter("wkT", [d, f], bf16, isOutput=False)
    wvT = nc.declare_dram_parameter("wvT", [d, f], bf16, isOutput=False)
    woT = nc.declare_dram_parameter("woT", [f, d], f32r, isOutput=False)
    out = nc.declare_dram_parameter("out", [d, n], f32, isOutput=True)

    xT_c = xT.rearrange("(c p) n -> c p n", p=P)
    wqT_c = wqT.rearrange("(c p) f -> c p f", p=P)
    wkT_c = wkT.rearrange("(c p) f -> c p f", p=P)
    wvT_c = wvT.rearrange("(c p) f -> c p f", p=P)
    woT_c = woT.rearrange("(c p) n -> c p n", p=P)

    with tile.TileContext(nc) as tc:
        with (
            tc.tile_pool(name="qkv", bufs=1) as qkv,
            tc.tile_pool(name="outT", bufs=1) as outp,
            # phase-2 pools created before the phase-1 pools so their
            # SBUF/PSUM ranges are disjoint: early heads' attention overlaps
            # the fc=1 projections with no pool-reuse serialization
            tc.tile_pool(name="pt", bufs=3) as ptp,
            tc.tile_pool(name="norm", bufs=2) as normp,
            tc.tile_pool(name="scps", bufs=2, space="PSUM") as scps,
            tc.tile_pool(name="avps", bufs=1, space="PSUM") as avps,
        ):
            QT_sb = qkv.tile([P, FC, n], f32r)
            # per-head K^T, zero-padded to a full 128-row stationary (head h
            # occupies partition rows po..po+hd, matching its rows in QT)
            KTz_sb = qkv.tile([P, hpc, n], f32r)
            V_sb = qkv.tile([P, NT, hpc, hd + 1], f32r)
            outT_sb = outp.tile([P, FC, n], f32r)
            # ones column of V_aug / zero fill of KTz: memset f32 consts, then
            # write via rounding DVE copies (direct memset on f32r fails
            # walrus codegen, and f32r matmul operands need rounding writers)
            ones_c = outp.tile([P, 1], f32)
            nc.vector.memset(ones_c[:], 1.0)
            nc.vector.tensor_copy(
                V_sb[:, :, :, hd : hd + 1],
                ones_c.to_broadcast([P, NT, hpc, 1]),
            )
            zero_c = outp.tile([P, 1], f32)
            nc.vector.memset(zero_c[:], 0.0)
            nc.vector.tensor_copy(
                KTz_sb[:], zero_c.to_broadcast([P, hpc, n])
            )

            def pass_begin():
                avoA = avps.tile([hd + 1, SCW], f32, tag="avoA", name="avoA")
                avoB = avps.tile([hd + 1, SCW], f32, tag="avoB", name="avoB")
                return avoA, avoB

            def emit_sc(hp, qq, kc):
                """Both heads' scores^T for k-chunk kc + ONE exp for the pair."""
                q0 = qq * SCW
                ks = slice(kc * P, (kc + 1) * P)
                qs = slice(q0, q0 + SCW)
                sc = scps.tile([P, 2, SCW], f32, tag="sc")
                nc.tensor.matmul(
                    sc[:, 0, :], KTz_sb[:, 2 * hp, ks], QT_sb[:, hp, qs],
                    start=True, stop=True,
                )
                nc.tensor.matmul(
                    sc[:, 1, :], KTz_sb[:, 2 * hp + 1, ks], QT_sb[:, hp, qs],
                    start=True, stop=True,
                )
                pt = ptp.tile([P, 2, SCW], f32r, tag="pt")
                nc.scalar.activation(
                    pt[:], sc[:], mybir.ActivationFunctionType.Exp,
                    scale=scale,
                )
                return pt

            def emit_av(avos, hp, kc, pt):
                avoA, avoB = avos
                nc.tensor.matmul(
                    avoA[:], V_sb[:, kc, 2 * hp, :], pt[:, 0, :],
                    start=(kc == 0), stop=(kc == NT - 1),
                )
                nc.tensor.matmul(
                    avoB[:], V_sb[:, kc, 2 * hp + 1, :], pt[:, 1, :],
                    start=(kc == 0), stop=(kc == NT - 1),
                )

            def pass_blocks(avos, hp, qq, pre_kc=None, mid=None):
                # skewed emission: scores(kc+1) before AV(kc) so the PE
                # stream never waits head-of-line on the exp for kc
                pend = None
                for kc in range(NT):
                    if mid is not None and kc == NT // 2:
                        mid()
                    if pre_kc is not None:
                        pre_kc(kc)
                    pt = emit_sc(hp, qq, kc)
                    if pend is not None:
                        emit_av(avos, hp, kc - 1, pend)
                    pend = pt
                emit_av(avos, hp, NT - 1, pend)

            def pass_end(avo, h, qq):
                """Free avo fast, then normalize rows 0..hd-1 by row hd (the
                softmax sums). reciprocal is single-lane-slow on a [1, SCW]
                row, so scatter the sums across partitions via a small SBUF
                DMA round-trip first."""
                fc = (h * hd) // P
                po = (h * hd) % P
                q0 = qq * SCW
                av_sb = normp.tile([hd + 1, SCW], f32, tag="av_sb")
                nc.vector.tensor_copy(av_sb[:], avo[:])
                rsh = normp.tile([P, SCW // P], f32, tag="rsh")
                nc.sync.dma_start(out=rsh[:], in_=av_sb[hd : hd + 1, :])
                rsh2 = normp.tile([P, SCW // P], f32, tag="rsh2")
                nc.vector.reciprocal(rsh2[:], rsh[:])
                recip = normp.tile([1, SCW], f32, tag="recip")
                nc.sync.dma_start(out=recip[:], in_=rsh2[:])
                bc = normp.tile([hd, SCW], f32, tag="bc")
                nc.gpsimd.partition_broadcast(bc[:], recip[:])
                nc.vector.tensor_mul(
                    outT_sb[po : po + hd, fc, q0 : q0 + SCW],
                    av_sb[0:hd, :],
                    bc[:],
                )

            def do_pass(hp, qq, pre_kc=None, mid=None):
                avos = pass_begin()
                pass_blocks(avos, hp, qq, pre_kc=pre_kc, mid=mid)
                pass_end(avos[0], 2 * hp, qq)
                pass_end(avos[1], 2 * hp + 1, qq)

            # ---- Phase 1 + head-pair 0 passes, emission-interleaved ----
            with (
                tc.tile_pool(name="xw", bufs=1) as xw,
                tc.tile_pool(name="p1ps", bufs=2, space="PSUM") as p1ps,
            ):
                xT_r = xw.tile([P, DC, n], bf16)
                wqT_r = xw.tile([P, DC, f], bf16)
                wkT_r = xw.tile([P, DC, f], bf16)
                wvT_r = xw.tile([P, DC, f], bf16)

                # wq + xT interleaved per chunk: QT matmuls stream right
                # behind them; wk/wv stream during QT/KT compute.
                for dc in range(DC):
                    nc.sync.dma_start(out=wqT_r[:, dc, :], in_=wqT_c[dc])
                    nc.sync.dma_start(out=xT_r[:, dc, :], in_=xT_c[dc])

                def proj_cols(w_sb, is_k, fc, qcp):
                    # dc-outer accumulation, one sub-stage of 2 held banks
                    # covering moving columns [qcp*QB, (qcp+2)*QB)
                    pss = [
                        p1ps.tile([P, QB], f32, tag="big", name=f"pj{g}")
                        for g in range(2)
                    ]
                    for dc in range(DC):
                        for j in range(2):
                            qc = qcp + j
                            nc.tensor.matmul(
                                pss[j][:],
                                w_sb[:, dc, fc * P : (fc + 1) * P],
                                xT_r[:, dc, qc * QB : (qc + 1) * QB],
                                start=(dc == 0),
                                stop=(dc == DC - 1),
                            )
                    for j in range(2):
                        qc = qcp + j
                        sl = slice(qc * QB, (qc + 1) * QB)
                        if is_k:
                            # rows 0:64 = head 2fc (po=0), rows 64:128 =
                            # head 2fc+1 (po=64); keep row alignment
                            nc.vector.tensor_copy(
                                KTz_sb[0:hd, 2 * fc, sl], pss[j][0:hd, :]
                            )
                            nc.vector.tensor_copy(
                                KTz_sb[hd : 2 * hd, 2 * fc + 1, sl],
                                pss[j][hd : 2 * hd, :],
                            )
                        else:
                            nc.vector.tensor_copy(QT_sb[:, fc, sl], pss[j][:])

                def v_tile(nt):
                    ps = p1ps.tile([P, QB], f32, tag="big", name="vps")
                    for dc in range(DC):
                        nc.tensor.matmul(
                            ps[:, 0:f],
                            xT_r[:, dc, nt * P : (nt + 1) * P],
                            wvT_r[:, dc, :],
                            start=(dc == 0),
                            stop=(dc == DC - 1),
                        )
                    nc.vector.tensor_copy(
                        V_sb[:, nt, :, 0:hd],
                        ps[:, 0:f].rearrange("p (h e) -> p h e", h=hpc),
                    )

                # wk needed right after the first k0 sub-stage; wv by the
                # first v_tile — both AFTER the xT stream in queue order so
                # they don't delay the projection-gating xT chunks
                for dc in range(DC):
                    nc.sync.dma_start(out=wkT_r[:, dc, :], in_=wkT_c[dc])
                for dc in range(DC):
                    nc.sync.dma_start(out=wvT_r[:, dc, :], in_=wvT_c[dc])
                # Emission order = scheduling priority. Minimal chain to the
                # first exp: QT cols of q-quarters 0/1, then K^T in column
                # sub-stages interleaved with the first pass's blocks (V
                # tiles interleaved per k-chunk they feed). Later projections
                # are emitted after the passes they should yield priority to,
                # so they fill the PE's exp-paced slack.
                proj_cols(wqT_r, False, 0, 0)  # QT fc0 cols 0:1024 (qq 0,1)
                avos0 = pass_begin()
                proj_cols(wkT_r, True, 0, 0)   # KTz fc0 cols 0:1024 (kc 0..7)
                pass_blocks(
                    avos0, 0, 0, pre_kc=v_tile,
                    mid=lambda: proj_cols(wkT_r, True, 0, 2),
                )
                pass_end(avos0[0], 0, 0)
                pass_end(avos0[1], 1, 0)
                do_pass(0, 1)
                proj_cols(wqT_r, False, 0, 2)  # QT fc0 cols for qq 2,3
                do_pass(0, 2)
                do_pass(0, 3)
                proj_cols(wqT_r, False, 1, 0)
                proj_cols(wqT_r, False, 1, 2)
                proj_cols(wkT_r, True, 1, 0)
                proj_cols(wkT_r, True, 1, 2)

            # ---- head-pair 1 passes + per-q-quarter output projection ----
            with (
                tc.tile_pool(name="wo", bufs=1) as wop,
                tc.tile_pool(name="wops", bufs=2, space="PSUM") as wopsp,
                tc.tile_pool(name="wosb", bufs=4) as wosbp,
            ):
                woT_sb = wop.tile([P, FC, d], f32r)
                for fc in range(FC):
                    nc.sync.dma_start(out=woT_sb[:, fc, :], in_=woT_c[fc])

                def wo_quarter(qq):
                    # output projection for q-quarter qq (woT stationary;
                    # emits partial^T [d, n])
                    q0 = qq * SCW
                    for do in range(d // P):
                        ps = wopsp.tile([P, SCW], f32, tag="wops")
                        for fc in range(FC):
                            nc.tensor.matmul(
                                ps[:],
                                woT_sb[:, fc, do * P : (do + 1) * P],
                                outT_sb[:, fc, q0 : q0 + SCW],
                                start=(fc == 0),
                                stop=(fc == FC - 1),
                            )
                        ob = wosbp.tile([P, SCW], f32, tag="ob")
                        nc.vector.tensor_copy(ob[:], ps[:])
                        nc.sync.dma_start(
                            out=out[do * P : (do + 1) * P, q0 : q0 + SCW],
                            in_=ob[:],
                        )

                do_pass(1, 0)
                wo_quarter(0)
                do_pass(1, 1)
                wo_quarter(1)
                do_pass(1, 2)
                wo_quarter(2)
                do_pass(1, 3)
                wo_quarter(3)
    nc.finalize()
    return nc


def make_in_maps(x, Wq, Wk, Wv, Wo):
    """Shard full inputs into per-core DRAM parameter maps."""
    import ml_dtypes

    bf16 = ml_dtypes.bfloat16
    x = np.asarray(x, dtype=np.float32)
    Wq = np.asarray(Wq, dtype=np.float32)
    Wk = np.asarray(Wk, dtype=np.float32)
    Wv = np.asarray(Wv, dtype=np.float32)
    Wo = np.asarray(Wo, dtype=np.float32)
    xTs = [np.ascontiguousarray(x[b].T).astype(bf16) for b in range(B)]
    WqT, WkT, WvT = Wq.T, Wk.T, Wv.T
    in_maps = []
    for c in range(N_CORES):
        b, g = c // (N_CORES // B), c % (N_CORES // B)
        fs = slice(g * F, (g + 1) * F)
        in_maps.append(
            {
                "xT": xTs[b],
                "wqT": np.ascontiguousarray(WqT[:, fs]).astype(bf16),
                "wkT": np.ascontiguousarray(WkT[:, fs]).astype(bf16),
                "wvT": np.ascontiguousarray(WvT[:, fs]).astype(bf16),
                "woT": np.ascontiguousarray(Wo[:, fs].T),
            }
        )
    return in_maps


_NC_CACHE = {}


def _enable_ldw_opt():
    """Flip walrus --enable-ldw-opt to true: consecutive matmuls sharing a
    stationary operand skip the redundant LDWEIGHTS reload."""
    import concourse.bass_utils as bu

    if getattr(bu, "_ldw_opt_patched", False):
        return
    orig = bu.run_command

    def patched(argv, **kw):
        argv = [
            "--enable-ldw-opt=true" if a == "--enable-ldw-opt=false" else a
            for a in argv
        ]
        return orig(argv, **kw)

    bu.run_command = patched
    bu._ldw_opt_patched = True


def run(x, Wq, Wk, Wv, Wo, trace=False):
    from concourse.bass_utils import run_bass_kernel_spmd

    # ldw-opt is left OFF: with the head-pair emission order consecutive
    # matmuls almost never share a stationary, and walrus rejects one of
    # the emitted LDWEIGHTS under --enable-ldw-opt=true.
    if "nc" not in _NC_CACHE:
        _NC_CACHE["nc"] = build_nc()
    nc = _NC_CACHE["nc"]
    in_maps = make_in_maps(x, Wq, Wk, Wv, Wo)
    res = run_bass_kernel_spmd(nc, in_maps, core_ids=list(range(N_CORES)), trace=trace)
    parts = [np.asarray(res.results[i]["out"]) for i in range(N_CORES)]
    gpb = N_CORES // B
    # per-core partials are transposed [d, n]: sum the group, then untranspose
    full = np.stack(
        [
            sum(parts[b * gpb + 1 : (b + 1) * gpb], parts[b * gpb]).T
            for b in range(B)
        ]
    )
    return np.ascontiguousarray(full, dtype=np.float32), res


def kernel(x, Wq, bq, Wk, bk, Wv, bv, Wo, bo):
    full, _ = run(x, Wq, Wk, Wv, Wo)
    return full
